# revision 13
# baseline (speedup 1.0000x reference)
"""Trainium2 Bass kernel: CategoricalActionHead.

reference semantics (per actor a):
    emb      = x_data[actors[a]]                       # [D]
    logits   = emb @ W.T + b                           # [C]
    logits   = where(mask==0, -inf, logits)
    logp     = log_softmax(logits)
    logprob  = logp[prev_actions[a]]
    entropy  = -sum_valid(p * logp)
    action   = prev_actions[a]

Sharding: data-parallel over the actor axis across 8 NeuronCores; x_data
(the 512MB table) and the small W/b are replicated per core.

Gather strategy: the dominant cost is gathering 32768 random 1KB rows per
core.  indirect_dma_start is descriptor-generation bound on the Q7 (~10ns
per row, ~330us/core).  Instead we use the CounterMachine dma_gather path
(~8ns/row on one SWDGE queue) spread over 4 SWDGE queues (~2ns/row
aggregate), which makes the gather HBM-transfer-bound.  dma_gather takes
int16 indices, so the host buckets each core's actors into 16 banks of
32768 rows (stable-sorted by bank; within-bank ascending for HBM
locality) and the kernel issues one dma_gather per bank, round-robin over
queues.  Bucketing permutes the actor order; the host permutes
mask/prev_actions into the same order (device layouts below) and
inverse-permutes the outputs.

Device layout: gathered row i of bank b lands at partition i%128, slot
i//128; global tile t = b*s_cap + i//128, q = t*128 + p.  Tiles are
processed in softmax groups of 16 (one PSUM bank = [128, 16*32] logits).
Host-side tensors for mask/pa/outputs are laid out [group, 128, 16(, 32)]
so every DMA is 128-partition contiguous.

Per tile pair: 4 PE transposes fill one PSUM bank [128, 512]; one copy
(alternating ACT/DVE) drains to SBUF; 4 accumulating matmuls produce
logits.  Masked log-softmax runs on [128, 16, 32] vector ops.

Numerics: exp() skips max-subtraction (|logits| <~ 1 since W std 0.01).
Masked lanes get logits - 30; exp(-30) ~ 1e-13 is invisible in f32 next
to the >= e^-1 valid lane.  The final logp adds (maskf-1)*30*1e38 (-inf
on masked lanes via overflow, exactly +0.0 on valid) to reproduce the
reference's -inf. Pad slots (bucket round-up) gather row 0 with mask 0
and are discarded by the host.
"""

import numpy as np

import concourse.bacc as bacc
import concourse.bass as bass
import concourse.tile as tile
from concourse import mybir

P = 128          # SBUF partitions
D = 256          # d_model
C = 32           # n_choice
G = 16           # tiles per softmax group (one PSUM bank)
N_CORES = 8
N_BANKS = 16
N_QUEUES = 4

F32 = mybir.dt.float32
I32 = mybir.dt.int32
I16 = mybir.dt.int16
ALU = mybir.AluOpType
ACTF = mybir.ActivationFunctionType
AX = mybir.AxisListType

MASK_NEG = 30.0


def _mid_bcast(ap, n):
    """[P, C] AP -> [P, n, C] AP with a 0-step middle dim."""
    return bass.AP(tensor=ap.tensor, offset=ap.offset, ap=[ap.ap[0], [0, n], ap.ap[1]])


def build_program(s_cap, bank_rows, banks=N_BANKS):
    """Per-core SPMD program.

    s_cap: 128-row slots per bank (bank gather = s_cap*128 rows);
    bank_rows: x_data rows per bank (bank_rows*banks total rows).
    """
    n_tiles = banks * s_cap
    assert n_tiles % G == 0
    n_groups = n_tiles // G
    gc = G * C            # 512 floats per partition per group
    cols = s_cap * P // 16  # int16 index columns per bank

    nc = bacc.Bacc(
        "TRN2", target_bir_lowering=False, debug=False, num_swdge_queues=N_QUEUES
    )

    x = nc.dram_tensor("x_data", [banks * bank_rows, D], F32, kind="ExternalInput").ap()
    wt = nc.dram_tensor("wt", [D, C], F32, kind="ExternalInput").ap()
    # pre-broadcast on host so no gpsimd DMA disturbs the SWDGE lane/queue
    # pairing (DMASW lanes round-robin; only the bank gathers may use them)
    bias_h = nc.dram_tensor("bias_fb", [P, G * C], F32, kind="ExternalInput").ap()
    iota_h = nc.dram_tensor("iota_b", [P, C], F32, kind="ExternalInput").ap()
    idx16 = nc.dram_tensor("idx16", [banks, P, cols], I16, kind="ExternalInput").ap()
    mask = nc.dram_tensor("mask", [n_groups, P, G, C], I32, kind="ExternalInput").ap()
    pa = nc.dram_tensor("pa", [n_groups, P, G], I32, kind="ExternalInput").ap()
    logp = nc.dram_tensor("logp", [n_groups, P, G, C], F32, kind="ExternalOutput").ap()
    logprob = nc.dram_tensor("logprob", [n_groups, P, G], F32, kind="ExternalOutput").ap()
    entropy = nc.dram_tensor("entropy", [n_groups, P, G], F32, kind="ExternalOutput").ap()

    with tile.TileContext(nc) as tc:
        with (
            tc.tile_pool(name="singles", bufs=1) as singles,
            tc.tile_pool(name="io", bufs=3) as io,
            tc.tile_pool(name="embp", bufs=2) as embp,
            tc.tile_pool(name="tpp", bufs=3) as tpp,
            tc.tile_pool(name="big", bufs=2) as big,
            tc.tile_pool(name="small", bufs=2) as small,
            tc.tile_pool(name="psum_l", bufs=2, space="PSUM") as psum_l,
            tc.tile_pool(name="psum_t", bufs=3, space="PSUM") as psum_t,
        ):
            from concourse.masks import make_identity

            identity = singles.tile([P, P], F32)
            make_identity(nc, identity[:])
            # wt_sb[p, h, c] = W.T[h*128 + p, c]
            wt_sb = singles.tile([P, 2, C], F32)
            nc.sync.dma_start(out=wt_sb[:], in_=wt.rearrange("(h p) c -> p h c", p=P))
            bias_fb = singles.tile([P, gc], F32)
            nc.sync.dma_start(out=bias_fb[:], in_=bias_h)
            iota_b = singles.tile([P, C], F32)
            nc.sync.dma_start(out=iota_b[:], in_=iota_h)
            idx_sb = singles.tile([P, banks, cols], I16)
            nc.sync.dma_start(out=idx_sb[:], in_=idx16.rearrange("b p c -> p b c"))

            # per-bank gathered embeddings, double buffered
            emb_tiles = {}

            def get_emb(b):
                if b not in emb_tiles:
                    t = embp.tile([P, s_cap, D], F32, tag="emb")
                    nc.gpsimd.dma_gather(
                        out_ap=t[:],
                        in_ap=x[b * bank_rows : (b + 1) * bank_rows, :],
                        idxs_ap=idx_sb[:, b, :],
                        num_idxs=s_cap * P,
                        num_idxs_reg=s_cap * P,
                        elem_size=D,
                        single_packet=False,
                        queue_num=b % N_QUEUES,
                    )
                    emb_tiles[b] = t
                return emb_tiles[b]

            for g in range(n_groups):
                mask_t = io.tile([P, gc], I32)
                nc.sync.dma_start(out=mask_t[:], in_=mask[g].rearrange("p t c -> p (t c)"))
                pa_t = io.tile([P, G], I32)
                nc.sync.dma_start(out=pa_t[:], in_=pa[g])

                ps_log = psum_l.tile([P, gc], F32)
                for tp_i in range(G // 2):  # tile pairs
                    embT4 = tpp.tile([P, 4, P], F32)
                    ps_t = psum_t.tile([P, 4, P], F32)
                    for half in range(4):  # (tile in pair, h)
                        t = g * G + tp_i * 2 + half // 2
                        b, si = divmod(t, s_cap)
                        h = half % 2
                        emb = get_emb(b)
                        nc.tensor.transpose(
                            out=ps_t[:, half, :],
                            in_=emb[:, si, h * P : (h + 1) * P],
                            identity=identity[:],
                        )
                    if tp_i % 2 == 0:
                        nc.scalar.copy(out=embT4[:], in_=ps_t[:])
                    else:
                        nc.vector.tensor_copy(out=embT4[:], in_=ps_t[:])
                    for half in range(4):
                        tt = tp_i * 2 + half // 2
                        h = half % 2
                        nc.tensor.matmul(
                            out=ps_log[:, tt * C : (tt + 1) * C],
                            lhsT=embT4[:, half, :],
                            rhs=wt_sb[:, h, :],
                            start=(h == 0),
                            stop=(h == 1),
                        )

                # ---- masked log-softmax over [P, G, C] ----
                maskf = big.tile([P, gc], F32)
                nc.vector.tensor_copy(out=maskf[:], in_=mask_t[:])
                # nb = (maskf - 1) * MASK_NEG   (0 on valid, -MASK_NEG on masked)
                nb = big.tile([P, gc], F32)
                nc.vector.tensor_scalar(
                    out=nb[:], in0=maskf[:], scalar1=-1.0, scalar2=MASK_NEG,
                    op0=ALU.add, op1=ALU.mult,
                )
                nb2 = big.tile([P, gc], F32)
                nc.vector.tensor_tensor(out=nb2[:], in0=nb[:], in1=bias_fb[:], op=ALU.add)
                lm = big.tile([P, gc], F32)
                nc.vector.tensor_tensor(out=lm[:], in0=ps_log[:], in1=nb2[:], op=ALU.add)
                e = big.tile([P, gc], F32)
                nc.scalar.activation(out=e[:], in_=lm[:], func=ACTF.Exp)
                s_sum = small.tile([P, G], F32)
                nc.vector.reduce_sum(
                    out=s_sum[:], in_=e[:].rearrange("p (t c) -> p t c", c=C), axis=AX.X
                )
                rs = small.tile([P, G], F32)
                nc.vector.reciprocal(out=rs[:], in_=s_sum[:])
                lse = small.tile([P, G], F32)
                nc.scalar.activation(out=lse[:], in_=s_sum[:], func=ACTF.Ln)
                logp_t = big.tile([P, gc], F32)
                nc.vector.tensor_tensor(
                    out=logp_t[:].rearrange("p (t c) -> p t c", c=C),
                    in0=lm[:].rearrange("p (t c) -> p t c", c=C),
                    in1=lse[:].to_broadcast((P, G, C)),
                    op=ALU.subtract,
                )
                p_ = big.tile([P, gc], F32)
                nc.vector.tensor_tensor(
                    out=p_[:].rearrange("p (t c) -> p t c", c=C),
                    in0=e[:].rearrange("p (t c) -> p t c", c=C),
                    in1=rs[:].to_broadcast((P, G, C)),
                    op=ALU.mult,
                )
                pl = big.tile([P, gc], F32)
                nc.vector.tensor_tensor(out=pl[:], in0=p_[:], in1=logp_t[:], op=ALU.mult)
                ent = small.tile([P, G], F32)
                nc.vector.tensor_reduce(
                    out=ent[:], in_=pl[:].rearrange("p (t c) -> p t c", c=C),
                    axis=AX.X, op=ALU.add, negate=True,
                )
                # masked -> -inf  ((-MASK_NEG)*1e38 overflows to -inf; valid: +0.0)
                lpo = big.tile([P, gc], F32)
                nc.vector.scalar_tensor_tensor(
                    out=lpo[:], in0=nb[:], scalar=1e38, in1=logp_t[:],
                    op0=ALU.mult, op1=ALU.add,
                )
                paf = small.tile([P, G], F32)
                nc.vector.tensor_copy(out=paf[:], in_=pa_t[:])
                oh = big.tile([P, gc], F32)
                nc.vector.tensor_tensor(
                    out=oh[:].rearrange("p (t c) -> p t c", c=C),
                    in0=_mid_bcast(iota_b[:], G),
                    in1=paf[:].to_broadcast((P, G, C)),
                    op=ALU.is_equal,
                )
                sel = big.tile([P, gc], F32)
                nc.vector.tensor_tensor(out=sel[:], in0=oh[:], in1=logp_t[:], op=ALU.mult)
                lp = small.tile([P, G], F32)
                nc.vector.reduce_sum(
                    out=lp[:], in_=sel[:].rearrange("p (t c) -> p t c", c=C), axis=AX.X
                )

                nc.sync.dma_start(out=logp[g].rearrange("p t c -> p (t c)"), in_=lpo[:])
                nc.sync.dma_start(out=logprob[g], in_=lp[:])
                nc.sync.dma_start(out=entropy[g], in_=ent[:])

    nc.compile()
    return nc


_PROGRAM_CACHE = {}


def _get_program(s_cap, bank_rows, banks=N_BANKS):
    key = (s_cap, bank_rows, banks)
    if key not in _PROGRAM_CACHE:
        _PROGRAM_CACHE[key] = build_program(s_cap, bank_rows, banks)
    return _PROGRAM_CACHE[key]


def _prepare_core(actors32, mask32, pa32, bank_rows, s_cap, banks=N_BANKS):
    """Bucket one core's actors by bank; build device-layout inputs and the
    flat device index (fl) of every actor for output unpermutation."""
    a_core = actors32.shape[0]
    shift = int(bank_rows).bit_length() - 1
    bank = actors32 >> shift
    order = np.argsort(bank, kind="stable")       # bucketed, ascending in-bank
    counts = np.bincount(bank, minlength=banks)
    starts = np.zeros(banks, np.int64)
    starts[1:] = np.cumsum(counts)[:-1]
    # within-bank position of each sorted actor
    pos_sorted = np.arange(a_core, dtype=np.int64) - starts[bank[order]]
    # device q of each sorted actor: tile = bank*s_cap + pos//128, p = pos%128
    bank_sorted = bank[order]
    t = bank_sorted * s_cap + (pos_sorted >> 7)
    p = pos_sorted & 127
    q = t * P + p
    # flat index in [n_groups, P, G] device layout: (g*P + p)*G + tt
    g = t // G
    tt = t % G
    fl = (g * P + p) * G + tt

    # int16 wrapped indices per bank, padded with 0
    local = (actors32 & (bank_rows - 1)).astype(np.int16)
    local_sorted = local[order]
    cols = s_cap * P // 16
    idx16 = np.zeros((banks, P, cols), np.int16)
    for b in range(banks):
        padded = np.zeros(s_cap * P, np.int16)
        padded[: counts[b]] = local_sorted[starts[b] : starts[b] + counts[b]]
        w = padded.reshape(cols, 16).T          # idx[i] at [i%16, i//16]
        idx16[b] = np.tile(w, (8, 1))

    n_tiles = banks * s_cap
    n_groups = n_tiles // G
    maskq = np.zeros((n_groups * P * G, C), np.int32)
    paq = np.zeros(n_groups * P * G, np.int32)
    maskq[fl] = mask32[order]
    paq[fl] = pa32[order]

    inv_fl = np.empty(a_core, np.int64)
    inv_fl[order] = fl
    return (
        idx16,
        maskq.reshape(n_groups, P, G, C),
        paq.reshape(n_groups, P, G),
        inv_fl,
    )


def _prepare_all(x_data, W, b, actors, mask, prev_actions):
    x_data = np.ascontiguousarray(np.asarray(x_data, dtype=np.float32))
    W = np.asarray(W, dtype=np.float32)
    b = np.ascontiguousarray(np.asarray(b, dtype=np.float32))
    actors32 = np.ascontiguousarray(np.asarray(actors).astype(np.int32))
    mask32 = np.ascontiguousarray(np.asarray(mask, dtype=np.int32))
    pa32 = np.ascontiguousarray(np.asarray(prev_actions).astype(np.int32))

    n_rows = x_data.shape[0]
    bank_rows = n_rows // N_BANKS
    a_total = actors32.shape[0]
    a_core = a_total // N_CORES
    shift = int(bank_rows).bit_length() - 1

    # s_cap = max bucket size over all (core, bank), in 128-row slots,
    # rounded so the tile count divides the softmax group size G
    max_count = 0
    for k in range(N_CORES):
        bk = actors32[k * a_core : (k + 1) * a_core] >> shift
        max_count = max(max_count, np.bincount(bk, minlength=N_BANKS).max())
    s_cap = int(-(-max_count // P))
    while (N_BANKS * s_cap) % G != 0:
        s_cap += 1

    wtT = np.ascontiguousarray(W.T)
    iota_b = np.tile(np.arange(C, dtype=np.float32), (P, 1))
    bias_fb = np.tile(b, (P, G))

    in_maps = []
    inv_fls = []
    for k in range(N_CORES):
        sl = slice(k * a_core, (k + 1) * a_core)
        idx16, maskq, paq, inv_fl = _prepare_core(
            actors32[sl], mask32[sl], pa32[sl], bank_rows, s_cap
        )
        in_maps.append(
            {
                "x_data": x_data,
                "wt": wtT,
                "bias_fb": np.ascontiguousarray(bias_fb, dtype=np.float32),
                "iota_b": iota_b,
                "idx16": idx16,
                "mask": maskq,
                "pa": paq,
            }
        )
        inv_fls.append(inv_fl)
    return in_maps, inv_fls, s_cap, bank_rows, a_core


def run_on_hw(x_data, W, b, actors, mask, prev_actions, trace=False):
    """Run the SPMD kernel on 8 NeuronCores; returns (outputs, BassKernelResults)."""
    from concourse.bass_utils import run_bass_kernel_spmd

    in_maps, inv_fls, s_cap, bank_rows, a_core = _prepare_all(
        x_data, W, b, actors, mask, prev_actions
    )
    nc = _get_program(s_cap, bank_rows)
    kres = run_bass_kernel_spmd(nc, in_maps, list(range(N_CORES)), trace=trace)
    res = kres.results
    logp = np.empty((N_CORES * a_core, C), np.float32)
    logprob = np.empty(N_CORES * a_core, np.float32)
    entropy = np.empty(N_CORES * a_core, np.float32)
    for k in range(N_CORES):
        sl = slice(k * a_core, (k + 1) * a_core)
        fl = inv_fls[k]
        logp[sl] = res[k]["logp"].reshape(-1, C)[fl]
        logprob[sl] = res[k]["logprob"].reshape(-1)[fl]
        entropy[sl] = res[k]["entropy"].reshape(-1)[fl]
    action = np.asarray(prev_actions).copy()
    return (action, logprob, entropy, logp), kres


def kernel(x_data, W, b, actors, mask, prev_actions, **_unused):
    outs, _ = run_on_hw(x_data, W, b, actors, mask, prev_actions, trace=False)
    return outs


# revision 14
# speedup vs baseline: 1.0913x; 1.0913x over previous
"""Trainium2 Bass kernel: CategoricalActionHead.

reference semantics (per actor a):
    emb      = x_data[actors[a]]                       # [D]
    logits   = emb @ W.T + b                           # [C]
    logits   = where(mask==0, -inf, logits)
    logp     = log_softmax(logits)
    logprob  = logp[prev_actions[a]]
    entropy  = -sum_valid(p * logp)
    action   = prev_actions[a]

Sharding: data-parallel over the actor axis across 8 NeuronCores; x_data
(the 512MB table) and the small W/b are replicated per core.

Gather strategy: the dominant cost is gathering 32768 random 1KB rows per
core.  indirect_dma_start is descriptor-generation bound on the Q7 (~10ns
per row, ~330us/core).  Instead we use the CounterMachine dma_gather path
(~8ns/row on one SWDGE queue) spread over 4 SWDGE queues (~2ns/row
aggregate), which makes the gather HBM-transfer-bound.  dma_gather takes
int16 indices, so the host buckets each core's actors into 16 banks of
32768 rows (stable-sorted by bank; within-bank ascending for HBM
locality) and the kernel issues one dma_gather per bank, round-robin over
queues.  Bucketing permutes the actor order; the host permutes
mask/prev_actions into the same order (device layouts below) and
inverse-permutes the outputs.

Device layout: gathered row i of bank b lands at partition i%128, slot
i//128; global tile t = b*s_cap + i//128, q = t*128 + p.  Tiles are
processed in softmax groups of 16 (one PSUM bank = [128, 16*32] logits).
Host-side tensors for mask/pa/outputs are laid out [group, 128, 16(, 32)]
so every DMA is 128-partition contiguous.

Per tile pair: 4 PE transposes fill one PSUM bank [128, 512]; one copy
(alternating ACT/DVE) drains to SBUF; 4 accumulating matmuls produce
logits.  Masked log-softmax runs on [128, 16, 32] vector ops.

Numerics: exp() skips max-subtraction (|logits| <~ 1 since W std 0.01).
Masked lanes get logits - 30; exp(-30) ~ 1e-13 is invisible in f32 next
to the >= e^-1 valid lane.  The final logp adds (maskf-1)*30*1e38 (-inf
on masked lanes via overflow, exactly +0.0 on valid) to reproduce the
reference's -inf. Pad slots (bucket round-up) gather row 0 with mask 0
and are discarded by the host.
"""

import numpy as np

import concourse.bacc as bacc
import concourse.bass as bass
import concourse.tile as tile
from concourse import mybir

P = 128          # SBUF partitions
D = 256          # d_model
C = 32           # n_choice
G = 16           # tiles per softmax group (one PSUM bank)
N_CORES = 8
N_BANKS = 16
N_QUEUES = 4

F32 = mybir.dt.float32
I32 = mybir.dt.int32
I16 = mybir.dt.int16
ALU = mybir.AluOpType
ACTF = mybir.ActivationFunctionType
AX = mybir.AxisListType

MASK_NEG = 30.0


def _mid_bcast(ap, n):
    """[P, C] AP -> [P, n, C] AP with a 0-step middle dim."""
    return bass.AP(tensor=ap.tensor, offset=ap.offset, ap=[ap.ap[0], [0, n], ap.ap[1]])


def build_program(s_cap, bank_rows, banks=N_BANKS):
    """Per-core SPMD program.

    s_cap: 128-row slots per bank (bank gather = s_cap*128 rows);
    bank_rows: x_data rows per bank (bank_rows*banks total rows).
    """
    n_tiles = banks * s_cap
    assert n_tiles % G == 0
    n_groups = n_tiles // G
    gc = G * C            # 512 floats per partition per group
    cols = s_cap * P // 16  # int16 index columns per bank

    nc = bacc.Bacc(
        "TRN2", target_bir_lowering=False, debug=False, num_swdge_queues=N_QUEUES
    )

    x = nc.dram_tensor("x_data", [banks * bank_rows, D], F32, kind="ExternalInput").ap()
    wt = nc.dram_tensor("wt", [D, C], F32, kind="ExternalInput").ap()
    # pre-broadcast on host so no gpsimd DMA disturbs the SWDGE lane/queue
    # pairing (DMASW lanes round-robin; only the bank gathers may use them)
    bias_h = nc.dram_tensor("bias_fb", [P, G * C], F32, kind="ExternalInput").ap()
    iota_h = nc.dram_tensor("iota_b", [P, C], F32, kind="ExternalInput").ap()
    idx16 = nc.dram_tensor("idx16", [banks, P, cols], I16, kind="ExternalInput").ap()
    mask = nc.dram_tensor("mask", [n_groups, P, G, C], I32, kind="ExternalInput").ap()
    pa = nc.dram_tensor("pa", [n_groups, P, G], I32, kind="ExternalInput").ap()
    logp = nc.dram_tensor("logp", [n_groups, P, G, C], F32, kind="ExternalOutput").ap()
    logprob = nc.dram_tensor("logprob", [n_groups, P, G], F32, kind="ExternalOutput").ap()
    entropy = nc.dram_tensor("entropy", [n_groups, P, G], F32, kind="ExternalOutput").ap()

    with tile.TileContext(nc) as tc:
        with (
            tc.tile_pool(name="singles", bufs=1) as singles,
            tc.tile_pool(name="io", bufs=3) as io,
            tc.tile_pool(name="embp", bufs=4) as embp,
            tc.tile_pool(name="tpp", bufs=3) as tpp,
            tc.tile_pool(name="big", bufs=2) as big,
            tc.tile_pool(name="small", bufs=2) as small,
            tc.tile_pool(name="psum_l", bufs=2, space="PSUM") as psum_l,
            tc.tile_pool(name="psum_t", bufs=3, space="PSUM") as psum_t,
        ):
            from concourse.masks import make_identity

            identity = singles.tile([P, P], F32)
            make_identity(nc, identity[:])
            # wt_sb[p, h, c] = W.T[h*128 + p, c]
            wt_sb = singles.tile([P, 2, C], F32)
            nc.sync.dma_start(out=wt_sb[:], in_=wt.rearrange("(h p) c -> p h c", p=P))
            bias_fb = singles.tile([P, gc], F32)
            nc.sync.dma_start(out=bias_fb[:], in_=bias_h)
            iota_b = singles.tile([P, C], F32)
            nc.sync.dma_start(out=iota_b[:], in_=iota_h)
            idx_sb = singles.tile([P, banks, cols], I16)
            nc.sync.dma_start(out=idx_sb[:], in_=idx16.rearrange("b p c -> p b c"))

            # per-bank gathered embeddings, double buffered
            emb_tiles = {}

            def get_emb(b):
                if b not in emb_tiles:
                    t = embp.tile([P, s_cap, D], F32, tag="emb")
                    nc.gpsimd.dma_gather(
                        out_ap=t[:],
                        in_ap=x[b * bank_rows : (b + 1) * bank_rows, :],
                        idxs_ap=idx_sb[:, b, :],
                        num_idxs=s_cap * P,
                        num_idxs_reg=s_cap * P,
                        elem_size=D,
                        single_packet=False,
                        queue_num=b % N_QUEUES,
                    )
                    emb_tiles[b] = t
                return emb_tiles[b]

            for g in range(n_groups):
                mask_t = io.tile([P, gc], I32)
                nc.sync.dma_start(out=mask_t[:], in_=mask[g].rearrange("p t c -> p (t c)"))
                pa_t = io.tile([P, G], I32)
                nc.sync.dma_start(out=pa_t[:], in_=pa[g])

                ps_log = psum_l.tile([P, gc], F32)
                for tp_i in range(G // 2):  # tile pairs
                    embT4 = tpp.tile([P, 4, P], F32)
                    ps_t = psum_t.tile([P, 4, P], F32)
                    for half in range(4):  # (tile in pair, h)
                        t = g * G + tp_i * 2 + half // 2
                        b, si = divmod(t, s_cap)
                        h = half % 2
                        emb = get_emb(b)
                        nc.tensor.transpose(
                            out=ps_t[:, half, :],
                            in_=emb[:, si, h * P : (h + 1) * P],
                            identity=identity[:],
                        )
                    if tp_i % 2 == 0:
                        nc.scalar.copy(out=embT4[:], in_=ps_t[:])
                    else:
                        nc.vector.tensor_copy(out=embT4[:], in_=ps_t[:])
                    for half in range(4):
                        tt = tp_i * 2 + half // 2
                        h = half % 2
                        nc.tensor.matmul(
                            out=ps_log[:, tt * C : (tt + 1) * C],
                            lhsT=embT4[:, half, :],
                            rhs=wt_sb[:, h, :],
                            start=(h == 0),
                            stop=(h == 1),
                        )

                # ---- masked log-softmax over [P, G, C] ----
                maskf = big.tile([P, gc], F32)
                nc.vector.tensor_copy(out=maskf[:], in_=mask_t[:])
                # nb = (maskf - 1) * MASK_NEG   (0 on valid, -MASK_NEG on masked)
                nb = big.tile([P, gc], F32)
                nc.vector.tensor_scalar(
                    out=nb[:], in0=maskf[:], scalar1=-1.0, scalar2=MASK_NEG,
                    op0=ALU.add, op1=ALU.mult,
                )
                nb2 = big.tile([P, gc], F32)
                nc.vector.tensor_tensor(out=nb2[:], in0=nb[:], in1=bias_fb[:], op=ALU.add)
                lm = big.tile([P, gc], F32)
                nc.vector.tensor_tensor(out=lm[:], in0=ps_log[:], in1=nb2[:], op=ALU.add)
                e = big.tile([P, gc], F32)
                nc.scalar.activation(out=e[:], in_=lm[:], func=ACTF.Exp)
                s_sum = small.tile([P, G], F32)
                nc.vector.reduce_sum(
                    out=s_sum[:], in_=e[:].rearrange("p (t c) -> p t c", c=C), axis=AX.X
                )
                rs = small.tile([P, G], F32)
                nc.vector.reciprocal(out=rs[:], in_=s_sum[:])
                lse = small.tile([P, G], F32)
                nc.scalar.activation(out=lse[:], in_=s_sum[:], func=ACTF.Ln)
                logp_t = big.tile([P, gc], F32)
                nc.vector.tensor_tensor(
                    out=logp_t[:].rearrange("p (t c) -> p t c", c=C),
                    in0=lm[:].rearrange("p (t c) -> p t c", c=C),
                    in1=lse[:].to_broadcast((P, G, C)),
                    op=ALU.subtract,
                )
                p_ = big.tile([P, gc], F32)
                nc.vector.tensor_tensor(
                    out=p_[:].rearrange("p (t c) -> p t c", c=C),
                    in0=e[:].rearrange("p (t c) -> p t c", c=C),
                    in1=rs[:].to_broadcast((P, G, C)),
                    op=ALU.mult,
                )
                pl = big.tile([P, gc], F32)
                nc.vector.tensor_tensor(out=pl[:], in0=p_[:], in1=logp_t[:], op=ALU.mult)
                ent = small.tile([P, G], F32)
                nc.vector.tensor_reduce(
                    out=ent[:], in_=pl[:].rearrange("p (t c) -> p t c", c=C),
                    axis=AX.X, op=ALU.add, negate=True,
                )
                # masked -> -inf  ((-MASK_NEG)*1e38 overflows to -inf; valid: +0.0)
                lpo = big.tile([P, gc], F32)
                nc.vector.scalar_tensor_tensor(
                    out=lpo[:], in0=nb[:], scalar=1e38, in1=logp_t[:],
                    op0=ALU.mult, op1=ALU.add,
                )
                paf = small.tile([P, G], F32)
                nc.vector.tensor_copy(out=paf[:], in_=pa_t[:])
                oh = big.tile([P, gc], F32)
                nc.vector.tensor_tensor(
                    out=oh[:].rearrange("p (t c) -> p t c", c=C),
                    in0=_mid_bcast(iota_b[:], G),
                    in1=paf[:].to_broadcast((P, G, C)),
                    op=ALU.is_equal,
                )
                sel = big.tile([P, gc], F32)
                nc.vector.tensor_tensor(out=sel[:], in0=oh[:], in1=logp_t[:], op=ALU.mult)
                lp = small.tile([P, G], F32)
                nc.vector.reduce_sum(
                    out=lp[:], in_=sel[:].rearrange("p (t c) -> p t c", c=C), axis=AX.X
                )

                nc.sync.dma_start(out=logp[g].rearrange("p t c -> p (t c)"), in_=lpo[:])
                nc.sync.dma_start(out=logprob[g], in_=lp[:])
                nc.sync.dma_start(out=entropy[g], in_=ent[:])

    nc.compile()
    return nc


_PROGRAM_CACHE = {}


def _get_program(s_cap, bank_rows, banks=N_BANKS):
    key = (s_cap, bank_rows, banks)
    if key not in _PROGRAM_CACHE:
        _PROGRAM_CACHE[key] = build_program(s_cap, bank_rows, banks)
    return _PROGRAM_CACHE[key]


def _prepare_core(actors32, mask32, pa32, bank_rows, s_cap, banks=N_BANKS):
    """Bucket one core's actors by bank; build device-layout inputs and the
    flat device index (fl) of every actor for output unpermutation."""
    a_core = actors32.shape[0]
    shift = int(bank_rows).bit_length() - 1
    bank = actors32 >> shift
    order = np.argsort(bank, kind="stable")       # bucketed, ascending in-bank
    counts = np.bincount(bank, minlength=banks)
    starts = np.zeros(banks, np.int64)
    starts[1:] = np.cumsum(counts)[:-1]
    # within-bank position of each sorted actor
    pos_sorted = np.arange(a_core, dtype=np.int64) - starts[bank[order]]
    # device q of each sorted actor: tile = bank*s_cap + pos//128, p = pos%128
    bank_sorted = bank[order]
    t = bank_sorted * s_cap + (pos_sorted >> 7)
    p = pos_sorted & 127
    q = t * P + p
    # flat index in [n_groups, P, G] device layout: (g*P + p)*G + tt
    g = t // G
    tt = t % G
    fl = (g * P + p) * G + tt

    # int16 wrapped indices per bank, padded with 0
    local = (actors32 & (bank_rows - 1)).astype(np.int16)
    local_sorted = local[order]
    cols = s_cap * P // 16
    idx16 = np.zeros((banks, P, cols), np.int16)
    for b in range(banks):
        padded = np.zeros(s_cap * P, np.int16)
        padded[: counts[b]] = local_sorted[starts[b] : starts[b] + counts[b]]
        w = padded.reshape(cols, 16).T          # idx[i] at [i%16, i//16]
        idx16[b] = np.tile(w, (8, 1))

    n_tiles = banks * s_cap
    n_groups = n_tiles // G
    maskq = np.zeros((n_groups * P * G, C), np.int32)
    paq = np.zeros(n_groups * P * G, np.int32)
    maskq[fl] = mask32[order]
    paq[fl] = pa32[order]

    inv_fl = np.empty(a_core, np.int64)
    inv_fl[order] = fl
    return (
        idx16,
        maskq.reshape(n_groups, P, G, C),
        paq.reshape(n_groups, P, G),
        inv_fl,
    )


def _prepare_all(x_data, W, b, actors, mask, prev_actions):
    x_data = np.ascontiguousarray(np.asarray(x_data, dtype=np.float32))
    W = np.asarray(W, dtype=np.float32)
    b = np.ascontiguousarray(np.asarray(b, dtype=np.float32))
    actors32 = np.ascontiguousarray(np.asarray(actors).astype(np.int32))
    mask32 = np.ascontiguousarray(np.asarray(mask, dtype=np.int32))
    pa32 = np.ascontiguousarray(np.asarray(prev_actions).astype(np.int32))

    n_rows = x_data.shape[0]
    bank_rows = n_rows // N_BANKS
    a_total = actors32.shape[0]
    a_core = a_total // N_CORES
    shift = int(bank_rows).bit_length() - 1

    # s_cap = max bucket size over all (core, bank), in 128-row slots,
    # rounded so the tile count divides the softmax group size G
    max_count = 0
    for k in range(N_CORES):
        bk = actors32[k * a_core : (k + 1) * a_core] >> shift
        max_count = max(max_count, np.bincount(bk, minlength=N_BANKS).max())
    s_cap = int(-(-max_count // P))
    while (N_BANKS * s_cap) % G != 0:
        s_cap += 1

    wtT = np.ascontiguousarray(W.T)
    iota_b = np.tile(np.arange(C, dtype=np.float32), (P, 1))
    bias_fb = np.tile(b, (P, G))

    in_maps = []
    inv_fls = []
    for k in range(N_CORES):
        sl = slice(k * a_core, (k + 1) * a_core)
        idx16, maskq, paq, inv_fl = _prepare_core(
            actors32[sl], mask32[sl], pa32[sl], bank_rows, s_cap
        )
        in_maps.append(
            {
                "x_data": x_data,
                "wt": wtT,
                "bias_fb": np.ascontiguousarray(bias_fb, dtype=np.float32),
                "iota_b": iota_b,
                "idx16": idx16,
                "mask": maskq,
                "pa": paq,
            }
        )
        inv_fls.append(inv_fl)
    return in_maps, inv_fls, s_cap, bank_rows, a_core


def run_on_hw(x_data, W, b, actors, mask, prev_actions, trace=False):
    """Run the SPMD kernel on 8 NeuronCores; returns (outputs, BassKernelResults)."""
    from concourse.bass_utils import run_bass_kernel_spmd

    in_maps, inv_fls, s_cap, bank_rows, a_core = _prepare_all(
        x_data, W, b, actors, mask, prev_actions
    )
    nc = _get_program(s_cap, bank_rows)
    kres = run_bass_kernel_spmd(nc, in_maps, list(range(N_CORES)), trace=trace)
    res = kres.results
    logp = np.empty((N_CORES * a_core, C), np.float32)
    logprob = np.empty(N_CORES * a_core, np.float32)
    entropy = np.empty(N_CORES * a_core, np.float32)
    for k in range(N_CORES):
        sl = slice(k * a_core, (k + 1) * a_core)
        fl = inv_fls[k]
        logp[sl] = res[k]["logp"].reshape(-1, C)[fl]
        logprob[sl] = res[k]["logprob"].reshape(-1)[fl]
        entropy[sl] = res[k]["entropy"].reshape(-1)[fl]
    action = np.asarray(prev_actions).copy()
    return (action, logprob, entropy, logp), kres


def kernel(x_data, W, b, actors, mask, prev_actions, **_unused):
    outs, _ = run_on_hw(x_data, W, b, actors, mask, prev_actions, trace=False)
    return outs
